# revision 10
# baseline (speedup 1.0000x reference)
"""Bass/Trainium2 kernel for a ragged-sequence CrossAttentionBlock.

Math (per reference):
  T = 16*196 packed tokens, D=512, H=8 heads of HD=64.
  q = (xq + pos) @ Wq + bq ; k = (xk + pos) @ Wk + bk ; v = xk @ Wv + bv
  block-diagonal attention over segments of channels[i]*196 tokens
  out = softmax(q k^T / 8) v  -> concat heads -> @ Wo + bo

Sharding: one head per NeuronCore (8 heads, 8 cores). Each core computes
its head's Q/K/V over all tokens, the per-segment attention, and its
head's slice of the output projection (out_h @ Wo[h*64:(h+1)*64, :]).
The host sums the 8 partial projections (bo is added on core 0).

Device layout: token axis is padded per segment to multiples of 128 and
kept TRANSPOSED ([feature, token]) throughout, so every matmul has its
contraction dim on partitions with no on-chip transposes of x. The +pos
fusion happens inside the QK projections by accumulating two matmuls
(W.T@x + W.T@pos) into the same PSUM bank. Softmax denominators come
from a ones-column appended to V. fp32r matmuls (full PE rate at free
dim >= 256, ~1e-4 accuracy).
"""

import os
import sys
import types
import time

import numpy as np

_D = 512
_HD = 64
_H = 8
_S = 196
_NCORES = 8

_prog_cache = {}


def _ensure_ntff_hook():
    """Register the NTFF profile hook that the agent image's antenv lacks."""
    if "antenv.axon_hooks" in sys.modules:
        return
    try:
        from trn_agent_boot.trn_boot import _ntff_profile_via_ctypes

        hook = _ntff_profile_via_ctypes("/opt/axon/libaxon_pjrt.so")
        mod = types.ModuleType("antenv.axon_hooks")
        mod.get_axon_ntff_profile_hook = lambda: hook
        sys.modules["antenv.axon_hooks"] = mod
    except Exception:
        pass


def _segments(channels):
    """Return (seg_len, seg_pad, seg_off, TP) for the padded token axis."""
    seg_len = [int(c) * _S for c in np.asarray(channels).tolist() if int(c) > 0]
    seg_pad = [(l + 127) // 128 * 128 for l in seg_len]
    TP = sum(seg_pad)
    TP = (TP + 511) // 512 * 512
    seg_off = []
    o = 0
    for p in seg_pad:
        seg_off.append(o)
        o += p
    return seg_len, seg_pad, seg_off, TP


def _build_program(seg_key):
    import concourse.bacc as bacc
    import concourse.tile as tile
    from concourse import mybir
    from concourse.masks import make_identity

    seg_len, seg_pad, seg_off, TP = seg_key[0], seg_key[1], seg_key[2], seg_key[3]
    f32 = mybir.dt.float32
    f32r = mybir.dt.float32r

    NTB = TP // 512  # 512-token blocks
    NKT = TP // 128  # 128-token tiles

    nc = bacc.Bacc("TRN2", target_bir_lowering=False, debug=False, num_devices=_NCORES)

    # Inputs: token-major-transposed, block layout [NTB, 512(feature), 512(token)]
    xqb = nc.dram_tensor("xqb", [NTB, _D, 512], f32, kind="ExternalInput")
    xkb = nc.dram_tensor("xkb", [NTB, _D, 512], f32, kind="ExternalInput")
    posb = nc.dram_tensor("posb", [NTB, _D, 512], f32, kind="ExternalInput")
    wq = nc.dram_tensor("wq", [_D, _HD], f32, kind="ExternalInput")
    wk = nc.dram_tensor("wk", [_D, _HD], f32, kind="ExternalInput")
    wv = nc.dram_tensor("wv", [_D, _HD], f32, kind="ExternalInput")
    wo = nc.dram_tensor("wo", [_HD, _D], f32, kind="ExternalInput")
    qkbias = nc.dram_tensor("qkbias", [_HD, 2], f32, kind="ExternalInput")
    vbias = nc.dram_tensor("vbias", [_HD, 1], f32, kind="ExternalInput")
    obiasT = nc.dram_tensor("obiasT", [128, 4], f32, kind="ExternalInput")
    nseg = len(seg_len)
    padbias = nc.dram_tensor("padbias", [128, nseg], f32, kind="ExternalInput")
    outT = nc.dram_tensor("outT", [_D, TP], f32, kind="ExternalOutput")

    with tile.TileContext(nc) as tc:
        with (
            tc.tile_pool(name="consts", bufs=1) as consts,
            tc.tile_pool(name="persist", bufs=1) as persist,
        ):
            # constants
            wq_sb = consts.tile([128, 4, _HD], f32r)
            wk_sb = consts.tile([128, 4, _HD], f32r)
            wv_sb = consts.tile([128, 4, _HD], f32r)
            nc.sync.dma_start(
                out=wq_sb, in_=wq.rearrange("(k p) d -> p k d", p=128).bitcast(f32r)
            )
            nc.sync.dma_start(
                out=wk_sb, in_=wk.rearrange("(k p) d -> p k d", p=128).bitcast(f32r)
            )
            nc.sync.dma_start(
                out=wv_sb, in_=wv.rearrange("(k p) d -> p k d", p=128).bitcast(f32r)
            )
            wo_sb = consts.tile([_HD, _D], f32r)
            nc.sync.dma_start(out=wo_sb, in_=wo[:, :].bitcast(f32r))
            qkb_sb = consts.tile([_HD, 2], f32)
            nc.sync.dma_start(out=qkb_sb, in_=qkbias[:, :])
            vb_sb = consts.tile([_HD, 1], f32)
            nc.sync.dma_start(out=vb_sb, in_=vbias[:, :])
            ob_sb = consts.tile([128, 4], f32)
            nc.sync.dma_start(out=ob_sb, in_=obiasT[:, :])
            pb_sb = consts.tile([128, nseg], f32)
            nc.sync.dma_start(out=pb_sb, in_=padbias[:, :])
            ident = consts.tile([128, 128], f32)
            make_identity(nc, ident)

            # persistent per-head tensors
            q_sb = persist.tile([_HD, TP], f32r)  # Q^T
            k_sb = persist.tile([_HD, TP], f32r)  # K^T
            v_sb = persist.tile([128, NKT, _HD + 1], f32r)  # V (normal) + ones col
            attn_sb = persist.tile([_HD, TP], f32r)  # attention output^T (normalized)

            # ---------------- Phase 1: projections ----------------
            with (
                tc.tile_pool(name="xin", bufs=3) as xin,
                tc.tile_pool(name="p1sb", bufs=3) as p1sb,
                tc.tile_pool(name="p1qk", bufs=2, space="PSUM") as p1qk,
                tc.tile_pool(name="p1vt", bufs=2, space="PSUM") as p1vt,
                tc.tile_pool(name="p1tr", bufs=2, space="PSUM") as p1tr,
            ):
                for tb in range(NTB):
                    ts = slice(tb * 512, (tb + 1) * 512)
                    xq_t = xin.tile([128, 4, 512], f32r, tag="xq")
                    xk_t = xin.tile([128, 4, 512], f32r, tag="xk")
                    pos_t = xin.tile([128, 4, 512], f32r, tag="pos")
                    nc.sync.dma_start(
                        out=xq_t,
                        in_=xqb[tb].rearrange("(k p) t -> p k t", p=128).bitcast(f32r),
                    )
                    nc.sync.dma_start(
                        out=xk_t,
                        in_=xkb[tb].rearrange("(k p) t -> p k t", p=128).bitcast(f32r),
                    )
                    nc.sync.dma_start(
                        out=pos_t,
                        in_=posb[tb].rearrange("(k p) t -> p k t", p=128).bitcast(f32r),
                    )

                    # Q^T and K^T accumulate in separate [64, 512] banks
                    q_ps = p1qk.tile([_HD, 512], f32, tag="qps")
                    k_ps = p1qk.tile([_HD, 512], f32, tag="kps")
                    for c in range(4):
                        nc.tensor.matmul(
                            q_ps, lhsT=wq_sb[:, c], rhs=xq_t[:, c],
                            start=(c == 0), stop=False,
                        )
                        nc.tensor.matmul(
                            q_ps, lhsT=wq_sb[:, c], rhs=pos_t[:, c],
                            start=False, stop=(c == 3),
                        )
                        nc.tensor.matmul(
                            k_ps, lhsT=wk_sb[:, c], rhs=xk_t[:, c],
                            start=(c == 0), stop=False,
                        )
                        nc.tensor.matmul(
                            k_ps, lhsT=wk_sb[:, c], rhs=pos_t[:, c],
                            start=False, stop=(c == 3),
                        )
                    # V^T accumulates in a [64, 512] bank
                    vt_ps = p1vt.tile([_HD, 512], f32)
                    for c in range(4):
                        nc.tensor.matmul(
                            vt_ps, lhsT=wv_sb[:, c], rhs=xk_t[:, c],
                            start=(c == 0), stop=(c == 3),
                        )

                    # copy out with bias
                    nc.scalar.activation(
                        out=q_sb[:, ts], in_=q_ps,
                        func=mybir.ActivationFunctionType.Identity,
                        bias=qkb_sb[:, 0:1],
                    )
                    nc.scalar.activation(
                        out=k_sb[:, ts], in_=k_ps,
                        func=mybir.ActivationFunctionType.Identity,
                        bias=qkb_sb[:, 1:2],
                    )
                    vt_sb = p1sb.tile([_HD, 512], f32, tag="vt")
                    nc.scalar.activation(
                        out=vt_sb, in_=vt_ps,
                        func=mybir.ActivationFunctionType.Identity,
                        bias=vb_sb,
                    )
                    # transpose V^T -> V normal, one 128-token tile at a time
                    for i in range(4):
                        kt = tb * 4 + i
                        tr_ps = p1tr.tile([128, _HD], f32)
                        nc.tensor.transpose(
                            tr_ps, vt_sb[:, i * 128:(i + 1) * 128], ident[0:64, 0:64]
                        )
                        nc.vector.tensor_copy(
                            out=v_sb[:, kt, 0:_HD], in_=tr_ps.bitcast(f32r)
                        )
                    nc.vector.memset(
                        v_sb[:, 4 * tb:4 * (tb + 1), _HD:_HD + 1].bitcast(f32), 1.0
                    )

            # ---------------- Phase 2: attention ----------------
            max_kt = max(p // 128 for p in seg_pad)
            max_l = max(seg_pad)
            with (
                tc.tile_pool(name="expp", bufs=2) as expp,
                tc.tile_pool(name="p2sb", bufs=3) as p2sb,
                tc.tile_pool(name="p2sc", bufs=3, space="PSUM") as p2sc,
                tc.tile_pool(name="p2o", bufs=2, space="PSUM") as p2o,
            ):
                for s in range(len(seg_len)):
                    off, L, real = seg_off[s], seg_pad[s], seg_len[s]
                    nkt = L // 128
                    qbs = [(qb0, min(512, L - qb0)) for qb0 in range(0, L, 512)]
                    ex = expp.tile([128, max_kt, max_l], f32r, tag="exp")
                    for kt in range(nkt):
                        klo = off + kt * 128
                        for (qb0, qbw) in qbs:
                            sc_ps = p2sc.tile([128, 512], f32, tag="sc")
                            nc.tensor.matmul(
                                sc_ps[:, 0:qbw],
                                lhsT=k_sb[:, klo:klo + 128],
                                rhs=q_sb[:, off + qb0:off + qb0 + qbw],
                                start=True, stop=True,
                            )
                            # padded key rows get bias -87 -> exp ~ 0
                            is_pad_tile = kt == nkt - 1 and real < L
                            nc.scalar.activation(
                                out=ex[:, kt, qb0:qb0 + qbw], in_=sc_ps[:, 0:qbw],
                                func=mybir.ActivationFunctionType.Exp,
                                scale=0.125,
                                bias=pb_sb[:, s:s + 1] if is_pad_tile else 0.0,
                            )
                    for (qb0, qbw) in qbs:
                        o_ps = p2o.tile([_HD + 1, 512], f32, tag="ops")
                        for kt in range(nkt):
                            nc.tensor.matmul(
                                o_ps[:, 0:qbw],
                                lhsT=v_sb[:, off // 128 + kt, :],
                                rhs=ex[:, kt, qb0:qb0 + qbw],
                                start=(kt == 0), stop=(kt == nkt - 1),
                            )
                        rec = p2sb.tile([1, 512], f32, tag="rec")
                        nc.vector.reciprocal(
                            out=rec[:, 0:qbw], in_=o_ps[_HD:_HD + 1, 0:qbw]
                        )
                        bc = p2sb.tile([_HD, 512], f32, tag="bc")
                        nc.gpsimd.partition_broadcast(
                            bc[:, 0:qbw], rec[:, 0:qbw]
                        )
                        nc.vector.tensor_mul(
                            attn_sb[:, off + qb0:off + qb0 + qbw],
                            o_ps[0:_HD, 0:qbw],
                            bc[:, 0:qbw],
                        )

            # ---------------- Phase 3: output projection ----------------
            with (
                tc.tile_pool(name="p3sb", bufs=3) as p3sb,
                tc.tile_pool(name="p3ps", bufs=3, space="PSUM") as p3ps,
            ):
                for ec in range(4):
                    for tb in range(NTB):
                        ts = slice(tb * 512, (tb + 1) * 512)
                        f_ps = p3ps.tile([128, 512], f32, tag="fin")
                        nc.tensor.matmul(
                            f_ps,
                            lhsT=wo_sb[:, ec * 128:(ec + 1) * 128],
                            rhs=attn_sb[:, ts],
                            start=True, stop=True,
                        )
                        f_sb = p3sb.tile([128, 512], f32, tag="fsb")
                        nc.vector.tensor_scalar_add(
                            f_sb, f_ps, ob_sb[:, ec:ec + 1]
                        )
                        nc.sync.dma_start(
                            out=outT[ec * 128:(ec + 1) * 128, ts], in_=f_sb
                        )

    nc.compile()
    return nc


def _prep_token_major(x, seg_len, seg_pad, seg_off, TP):
    """[T, D] f32 -> blocked transposed [NTB, D, 512] with per-segment padding."""
    xp = np.zeros((TP, _D), dtype=np.float32)
    o = 0
    for l, p, off in zip(seg_len, seg_pad, seg_off):
        xp[off:off + l] = x[o:o + l]
        o += l
    xt = np.ascontiguousarray(xp.T)  # [D, TP]
    return np.ascontiguousarray(
        xt.reshape(_D, TP // 512, 512).transpose(1, 0, 2)
    )  # [NTB, D, 512]


def kernel(x_query, x_keyval, pos, channels, Wq, bq, Wk, bk, Wv, bv, Wo, bo,
           _trace=False, _trace_cores=None):
    _ensure_ntff_hook()
    import concourse.bass_utils as bu

    bu.upload_artifacts = lambda tmpdir: tmpdir  # no S3 egress from here

    x_query = np.asarray(x_query, dtype=np.float32)
    x_keyval = np.asarray(x_keyval, dtype=np.float32)
    pos = np.asarray(pos, dtype=np.float32)
    channels = np.asarray(channels)
    Wq, bq = np.asarray(Wq, np.float32), np.asarray(bq, np.float32)
    Wk, bk = np.asarray(Wk, np.float32), np.asarray(bk, np.float32)
    Wv, bv = np.asarray(Wv, np.float32), np.asarray(bv, np.float32)
    Wo, bo = np.asarray(Wo, np.float32), np.asarray(bo, np.float32)

    C, S, D = x_query.shape
    seg_len, seg_pad, seg_off, TP = _segments(channels)
    assert sum(seg_len) == C * S, "channels inconsistent with batch dim"

    seg_key = (tuple(seg_len), tuple(seg_pad), tuple(seg_off), TP)
    if seg_key not in _prog_cache:
        _prog_cache[seg_key] = _build_program(seg_key)
    nc = _prog_cache[seg_key]

    xqb = _prep_token_major(x_query.reshape(-1, D), seg_len, seg_pad, seg_off, TP)
    xkb = _prep_token_major(x_keyval.reshape(-1, D), seg_len, seg_pad, seg_off, TP)
    posb = _prep_token_major(pos.reshape(-1, D), seg_len, seg_pad, seg_off, TP)

    # per-segment pad bias: -87 on padded key rows of the segment's last k-tile
    nseg = len(seg_len)
    padbias = np.zeros((128, nseg), dtype=np.float32)
    for s in range(nseg):
        plo = seg_len[s] - (seg_pad[s] // 128 - 1) * 128
        if plo < 128:
            padbias[plo:, s] = -87.0

    in_maps = []
    for h in range(_NCORES):
        sl = slice(h * _HD, (h + 1) * _HD)
        ob = bo if h == 0 else np.zeros_like(bo)
        in_maps.append({
            "xqb": xqb,
            "xkb": xkb,
            "posb": posb,
            "wq": np.ascontiguousarray(Wq[:, sl]),
            "wk": np.ascontiguousarray(Wk[:, sl]),
            "wv": np.ascontiguousarray(Wv[:, sl]),
            "wo": np.ascontiguousarray(Wo[sl, :]),
            "qkbias": np.ascontiguousarray(np.stack([bq[sl], bk[sl]], axis=1)),
            "vbias": np.ascontiguousarray(bv[sl].reshape(_HD, 1)),
            "obiasT": np.ascontiguousarray(ob.reshape(4, 128).T),
            "padbias": padbias,
        })

    from concourse.bass_utils import run_bass_kernel_spmd

    kwargs = {}
    if _trace:
        kwargs["trace"] = True
        if _trace_cores is not None:
            kwargs["trace_cores"] = _trace_cores
    res = run_bass_kernel_spmd(nc, in_maps, list(range(_NCORES)), **kwargs)

    acc = np.zeros((_D, TP), dtype=np.float64)
    for h in range(_NCORES):
        acc += res.results[h]["outT"]
    outT = acc.astype(np.float32)

    # unpad + transpose back
    out = np.empty((C * S, D), dtype=np.float32)
    o = 0
    for l, off in zip(seg_len, seg_off):
        out[o:o + l] = outT[:, off:off + l].T
        o += l
    out = out.reshape(C, S, D)

    if _trace:
        kernel._last_exec_time_ns = res.exec_time_ns
        kernel._last_trace = (
            res.instructions_and_trace[1] if res.instructions_and_trace else None
        )
    return out


# revision 11
# speedup vs baseline: 1.1834x; 1.1834x over previous
"""Bass/Trainium2 kernel for a ragged-sequence CrossAttentionBlock.

Math (per reference):
  T = 16*196 packed tokens, D=512, H=8 heads of HD=64.
  q = (xq + pos) @ Wq + bq ; k = (xk + pos) @ Wk + bk ; v = xk @ Wv + bv
  block-diagonal attention over segments of channels[i]*196 tokens
  out = softmax(q k^T / 8) v  -> concat heads -> @ Wo + bo

Sharding: one head per NeuronCore (8 heads, 8 cores). Each core computes
its head's Q/K/V over all tokens, the per-segment attention, and its
head's slice of the output projection (out_h @ Wo[h*64:(h+1)*64, :]).
The host sums the 8 partial projections (bo is added on core 0).

Device layout: token axis is padded per segment to multiples of 128 and
kept TRANSPOSED ([feature, token]) throughout, so every matmul has its
contraction dim on partitions with no on-chip transposes of x. The +pos
fusion happens inside the QK projections by accumulating two matmuls
(W.T@x + W.T@pos) into the same PSUM bank. Softmax denominators come
from a ones-column appended to V. fp32r matmuls (full PE rate at free
dim >= 256, ~1e-4 accuracy).
"""

import os
import sys
import types
import time

import numpy as np
import ml_dtypes

_D = 512
_HD = 64
_H = 8
_S = 196
_NCORES = 8

_prog_cache = {}


def _ensure_ntff_hook():
    """Register the NTFF profile hook that the agent image's antenv lacks."""
    if "antenv.axon_hooks" in sys.modules:
        return
    try:
        from trn_agent_boot.trn_boot import _ntff_profile_via_ctypes

        hook = _ntff_profile_via_ctypes("/opt/axon/libaxon_pjrt.so")
        mod = types.ModuleType("antenv.axon_hooks")
        mod.get_axon_ntff_profile_hook = lambda: hook
        sys.modules["antenv.axon_hooks"] = mod
    except Exception:
        pass


def _segments(channels):
    """Return (seg_len, seg_pad, seg_off, TP) for the padded token axis."""
    seg_len = [int(c) * _S for c in np.asarray(channels).tolist() if int(c) > 0]
    seg_pad = [(l + 127) // 128 * 128 for l in seg_len]
    TP = sum(seg_pad)
    TP = (TP + 511) // 512 * 512
    seg_off = []
    o = 0
    for p in seg_pad:
        seg_off.append(o)
        o += p
    return seg_len, seg_pad, seg_off, TP


def _build_program(seg_key):
    import concourse.bacc as bacc
    import concourse.tile as tile
    from concourse import mybir
    from concourse.masks import make_identity

    seg_len, seg_pad, seg_off, TP = seg_key[0], seg_key[1], seg_key[2], seg_key[3]
    f32 = mybir.dt.float32
    bf16 = mybir.dt.bfloat16

    NTB = TP // 512  # 512-token blocks
    NKT = TP // 128  # 128-token tiles

    nc = bacc.Bacc("TRN2", target_bir_lowering=False, debug=False, num_devices=_NCORES)

    # Inputs: token-major-transposed, block layout [NTB, 512(feature), 512(token)]
    xqb = nc.dram_tensor("xqb", [NTB, _D, 512], bf16, kind="ExternalInput")
    xkb = nc.dram_tensor("xkb", [NTB, _D, 512], bf16, kind="ExternalInput")
    posb = nc.dram_tensor("posb", [NTB, _D, 512], bf16, kind="ExternalInput")
    wq = nc.dram_tensor("wq", [_D, _HD], bf16, kind="ExternalInput")
    wk = nc.dram_tensor("wk", [_D, _HD], bf16, kind="ExternalInput")
    wv = nc.dram_tensor("wv", [_D, _HD], bf16, kind="ExternalInput")
    wo = nc.dram_tensor("wo", [_HD, _D], bf16, kind="ExternalInput")
    qkbias = nc.dram_tensor("qkbias", [_HD, 2], f32, kind="ExternalInput")
    vbias = nc.dram_tensor("vbias", [_HD, 1], f32, kind="ExternalInput")
    obiasT = nc.dram_tensor("obiasT", [128, 4], f32, kind="ExternalInput")
    nseg = len(seg_len)
    padbias = nc.dram_tensor("padbias", [128, nseg], f32, kind="ExternalInput")
    outT = nc.dram_tensor("outT", [_D, TP], f32, kind="ExternalOutput")

    with tile.TileContext(nc) as tc:
        with (
            tc.tile_pool(name="consts", bufs=1) as consts,
            tc.tile_pool(name="persist", bufs=1) as persist,
        ):
            # constants
            wq_sb = consts.tile([128, 4, _HD], bf16)
            wk_sb = consts.tile([128, 4, _HD], bf16)
            wv_sb = consts.tile([128, 4, _HD], bf16)
            nc.sync.dma_start(
                out=wq_sb, in_=wq.rearrange("(k p) d -> p k d", p=128)
            )
            nc.sync.dma_start(
                out=wk_sb, in_=wk.rearrange("(k p) d -> p k d", p=128)
            )
            nc.sync.dma_start(
                out=wv_sb, in_=wv.rearrange("(k p) d -> p k d", p=128)
            )
            wo_sb = consts.tile([_HD, _D], bf16)
            nc.sync.dma_start(out=wo_sb, in_=wo[:, :])
            qkb_sb = consts.tile([_HD, 2], f32)
            nc.sync.dma_start(out=qkb_sb, in_=qkbias[:, :])
            vb_sb = consts.tile([_HD, 1], f32)
            nc.sync.dma_start(out=vb_sb, in_=vbias[:, :])
            ob_sb = consts.tile([128, 4], f32)
            nc.sync.dma_start(out=ob_sb, in_=obiasT[:, :])
            pb_sb = consts.tile([128, nseg], f32)
            nc.sync.dma_start(out=pb_sb, in_=padbias[:, :])
            ident = consts.tile([128, 128], bf16)
            make_identity(nc, ident)

            # persistent per-head tensors
            q_sb = persist.tile([_HD, TP], bf16)  # Q^T
            k_sb = persist.tile([_HD, TP], bf16)  # K^T
            v_sb = persist.tile([128, NKT, _HD + 1], bf16)  # V (normal) + ones col
            attn_sb = persist.tile([_HD, TP], bf16)  # attention output^T (normalized)

            # ---------------- Phase 1: projections ----------------
            with (
                tc.tile_pool(name="xin", bufs=3) as xin,
                tc.tile_pool(name="p1sb", bufs=3) as p1sb,
                tc.tile_pool(name="p1qk", bufs=2, space="PSUM") as p1qk,
                tc.tile_pool(name="p1vt", bufs=2, space="PSUM") as p1vt,
                tc.tile_pool(name="p1tr", bufs=2, space="PSUM") as p1tr,
            ):
                for tb in range(NTB):
                    ts = slice(tb * 512, (tb + 1) * 512)
                    xq_t = xin.tile([128, 4, 512], bf16, tag="xq")
                    xk_t = xin.tile([128, 4, 512], bf16, tag="xk")
                    pos_t = xin.tile([128, 4, 512], bf16, tag="pos")
                    nc.sync.dma_start(
                        out=xq_t,
                        in_=xqb[tb].rearrange("(k p) t -> p k t", p=128),
                    )
                    nc.sync.dma_start(
                        out=xk_t,
                        in_=xkb[tb].rearrange("(k p) t -> p k t", p=128),
                    )
                    nc.sync.dma_start(
                        out=pos_t,
                        in_=posb[tb].rearrange("(k p) t -> p k t", p=128),
                    )

                    # Q^T and K^T accumulate in separate [64, 512] banks
                    q_ps = p1qk.tile([_HD, 512], f32, tag="qps")
                    k_ps = p1qk.tile([_HD, 512], f32, tag="kps")
                    for c in range(4):
                        nc.tensor.matmul(
                            q_ps, lhsT=wq_sb[:, c], rhs=xq_t[:, c],
                            start=(c == 0), stop=False,
                        )
                        nc.tensor.matmul(
                            q_ps, lhsT=wq_sb[:, c], rhs=pos_t[:, c],
                            start=False, stop=(c == 3),
                        )
                        nc.tensor.matmul(
                            k_ps, lhsT=wk_sb[:, c], rhs=xk_t[:, c],
                            start=(c == 0), stop=False,
                        )
                        nc.tensor.matmul(
                            k_ps, lhsT=wk_sb[:, c], rhs=pos_t[:, c],
                            start=False, stop=(c == 3),
                        )
                    # V^T accumulates in a [64, 512] bank
                    vt_ps = p1vt.tile([_HD, 512], f32)
                    for c in range(4):
                        nc.tensor.matmul(
                            vt_ps, lhsT=wv_sb[:, c], rhs=xk_t[:, c],
                            start=(c == 0), stop=(c == 3),
                        )

                    # copy out with bias
                    nc.scalar.activation(
                        out=q_sb[:, ts], in_=q_ps,
                        func=mybir.ActivationFunctionType.Identity,
                        bias=qkb_sb[:, 0:1],
                    )
                    nc.scalar.activation(
                        out=k_sb[:, ts], in_=k_ps,
                        func=mybir.ActivationFunctionType.Identity,
                        bias=qkb_sb[:, 1:2],
                    )
                    vt_sb = p1sb.tile([_HD, 512], bf16, tag="vt")
                    nc.scalar.activation(
                        out=vt_sb, in_=vt_ps,
                        func=mybir.ActivationFunctionType.Identity,
                        bias=vb_sb,
                    )
                    # transpose V^T -> V normal, one 128-token tile at a time
                    for i in range(4):
                        kt = tb * 4 + i
                        tr_ps = p1tr.tile([128, _HD], bf16)
                        nc.tensor.transpose(
                            tr_ps, vt_sb[:, i * 128:(i + 1) * 128], ident[0:64, 0:64]
                        )
                        nc.vector.tensor_copy(
                            out=v_sb[:, kt, 0:_HD], in_=tr_ps
                        )
                    nc.vector.memset(
                        v_sb[:, 4 * tb:4 * (tb + 1), _HD:_HD + 1], 1.0
                    )

            # ---------------- Phase 2: attention ----------------
            max_kt = max(p // 128 for p in seg_pad)
            max_l = max(seg_pad)
            with (
                tc.tile_pool(name="expp", bufs=2) as expp,
                tc.tile_pool(name="p2sb", bufs=3) as p2sb,
                tc.tile_pool(name="p2sc", bufs=3, space="PSUM") as p2sc,
                tc.tile_pool(name="p2o", bufs=2, space="PSUM") as p2o,
            ):
                for s in range(len(seg_len)):
                    off, L, real = seg_off[s], seg_pad[s], seg_len[s]
                    nkt = L // 128
                    qbs = [(qb0, min(512, L - qb0)) for qb0 in range(0, L, 512)]
                    ex = expp.tile([128, max_kt, max_l], bf16, tag="exp")
                    for kt in range(nkt):
                        klo = off + kt * 128
                        for (qb0, qbw) in qbs:
                            sc_ps = p2sc.tile([128, 512], f32, tag="sc")
                            nc.tensor.matmul(
                                sc_ps[:, 0:qbw],
                                lhsT=k_sb[:, klo:klo + 128],
                                rhs=q_sb[:, off + qb0:off + qb0 + qbw],
                                start=True, stop=True,
                            )
                            # padded key rows get bias -87 -> exp ~ 0
                            is_pad_tile = kt == nkt - 1 and real < L
                            nc.scalar.activation(
                                out=ex[:, kt, qb0:qb0 + qbw], in_=sc_ps[:, 0:qbw],
                                func=mybir.ActivationFunctionType.Exp,
                                scale=0.125,
                                bias=pb_sb[:, s:s + 1] if is_pad_tile else 0.0,
                            )
                    for (qb0, qbw) in qbs:
                        o_ps = p2o.tile([_HD + 1, 512], f32, tag="ops")
                        for kt in range(nkt):
                            nc.tensor.matmul(
                                o_ps[:, 0:qbw],
                                lhsT=v_sb[:, off // 128 + kt, :],
                                rhs=ex[:, kt, qb0:qb0 + qbw],
                                start=(kt == 0), stop=(kt == nkt - 1),
                            )
                        sums = p2sb.tile([1, 512], f32, tag="sums")
                        nc.scalar.copy(out=sums[:, 0:qbw], in_=o_ps[_HD:_HD + 1, 0:qbw])
                        bc = p2sb.tile([_HD, 512], f32, tag="bc")
                        nc.gpsimd.partition_broadcast(bc[:, 0:qbw], sums[:, 0:qbw])
                        rec = p2sb.tile([_HD, 512], f32, tag="rec")
                        nc.vector.reciprocal(out=rec[:, 0:qbw], in_=bc[:, 0:qbw])
                        nc.vector.tensor_mul(
                            attn_sb[:, off + qb0:off + qb0 + qbw],
                            o_ps[0:_HD, 0:qbw],
                            rec[:, 0:qbw],
                        )

            # ---------------- Phase 3: output projection ----------------
            with (
                tc.tile_pool(name="p3sb", bufs=3) as p3sb,
                tc.tile_pool(name="p3ps", bufs=3, space="PSUM") as p3ps,
            ):
                for ec in range(4):
                    for tb in range(NTB):
                        ts = slice(tb * 512, (tb + 1) * 512)
                        f_ps = p3ps.tile([128, 512], f32, tag="fin")
                        nc.tensor.matmul(
                            f_ps,
                            lhsT=wo_sb[:, ec * 128:(ec + 1) * 128],
                            rhs=attn_sb[:, ts],
                            start=True, stop=True,
                        )
                        f_sb = p3sb.tile([128, 512], f32, tag="fsb")
                        nc.vector.tensor_scalar_add(
                            f_sb, f_ps, ob_sb[:, ec:ec + 1]
                        )
                        nc.sync.dma_start(
                            out=outT[ec * 128:(ec + 1) * 128, ts], in_=f_sb
                        )

    nc.compile()
    return nc


def _prep_token_major(x, seg_len, seg_pad, seg_off, TP):
    """[T, D] f32 -> blocked transposed [NTB, D, 512] with per-segment padding."""
    xp = np.zeros((TP, _D), dtype=np.float32)
    o = 0
    for l, p, off in zip(seg_len, seg_pad, seg_off):
        xp[off:off + l] = x[o:o + l]
        o += l
    xt = np.ascontiguousarray(xp.T)  # [D, TP]
    return np.ascontiguousarray(
        xt.reshape(_D, TP // 512, 512).transpose(1, 0, 2)
    )  # [NTB, D, 512]


def kernel(x_query, x_keyval, pos, channels, Wq, bq, Wk, bk, Wv, bv, Wo, bo,
           _trace=False, _trace_cores=None):
    _ensure_ntff_hook()
    import concourse.bass_utils as bu

    bu.upload_artifacts = lambda tmpdir: tmpdir  # no S3 egress from here

    x_query = np.asarray(x_query, dtype=np.float32)
    x_keyval = np.asarray(x_keyval, dtype=np.float32)
    pos = np.asarray(pos, dtype=np.float32)
    channels = np.asarray(channels)
    Wq, bq = np.asarray(Wq, np.float32), np.asarray(bq, np.float32)
    Wk, bk = np.asarray(Wk, np.float32), np.asarray(bk, np.float32)
    Wv, bv = np.asarray(Wv, np.float32), np.asarray(bv, np.float32)
    Wo, bo = np.asarray(Wo, np.float32), np.asarray(bo, np.float32)

    C, S, D = x_query.shape
    seg_len, seg_pad, seg_off, TP = _segments(channels)
    assert sum(seg_len) == C * S, "channels inconsistent with batch dim"

    seg_key = (tuple(seg_len), tuple(seg_pad), tuple(seg_off), TP)
    if seg_key not in _prog_cache:
        _prog_cache[seg_key] = _build_program(seg_key)
    nc = _prog_cache[seg_key]

    bf = ml_dtypes.bfloat16
    xqb = _prep_token_major(x_query.reshape(-1, D), seg_len, seg_pad, seg_off, TP).astype(bf)
    xkb = _prep_token_major(x_keyval.reshape(-1, D), seg_len, seg_pad, seg_off, TP).astype(bf)
    posb = _prep_token_major(pos.reshape(-1, D), seg_len, seg_pad, seg_off, TP).astype(bf)

    # per-segment pad bias: -87 on padded key rows of the segment's last k-tile
    nseg = len(seg_len)
    padbias = np.zeros((128, nseg), dtype=np.float32)
    for s in range(nseg):
        plo = seg_len[s] - (seg_pad[s] // 128 - 1) * 128
        if plo < 128:
            padbias[plo:, s] = -87.0

    in_maps = []
    for h in range(_NCORES):
        sl = slice(h * _HD, (h + 1) * _HD)
        ob = bo if h == 0 else np.zeros_like(bo)
        in_maps.append({
            "xqb": xqb,
            "xkb": xkb,
            "posb": posb,
            "wq": np.ascontiguousarray(Wq[:, sl]).astype(bf),
            "wk": np.ascontiguousarray(Wk[:, sl]).astype(bf),
            "wv": np.ascontiguousarray(Wv[:, sl]).astype(bf),
            "wo": np.ascontiguousarray(Wo[sl, :]).astype(bf),
            "qkbias": np.ascontiguousarray(np.stack([bq[sl], bk[sl]], axis=1)),
            "vbias": np.ascontiguousarray(bv[sl].reshape(_HD, 1)),
            "obiasT": np.ascontiguousarray(ob.reshape(4, 128).T),
            "padbias": padbias,
        })

    from concourse.bass_utils import run_bass_kernel_spmd

    kwargs = {}
    if _trace:
        kwargs["trace"] = True
        if _trace_cores is not None:
            kwargs["trace_cores"] = _trace_cores
    res = run_bass_kernel_spmd(nc, in_maps, list(range(_NCORES)), **kwargs)

    acc = np.zeros((_D, TP), dtype=np.float64)
    for h in range(_NCORES):
        acc += res.results[h]["outT"]
    outT = acc.astype(np.float32)

    # unpad + transpose back
    out = np.empty((C * S, D), dtype=np.float32)
    o = 0
    for l, off in zip(seg_len, seg_off):
        out[o:o + l] = outT[:, off:off + l].T
        o += l
    out = out.reshape(C, S, D)

    if _trace:
        kernel._last_exec_time_ns = res.exec_time_ns
        kernel._last_trace = (
            res.instructions_and_trace[1] if res.instructions_and_trace else None
        )
    return out


# revision 12
# speedup vs baseline: 1.1944x; 1.0093x over previous
"""Bass/Trainium2 kernel for a ragged-sequence CrossAttentionBlock.

Math (per reference):
  T = 16*196 packed tokens, D=512, H=8 heads of HD=64.
  q = (xq + pos) @ Wq + bq ; k = (xk + pos) @ Wk + bk ; v = xk @ Wv + bv
  block-diagonal attention over segments of channels[i]*196 tokens
  out = softmax(q k^T / 8) v  -> concat heads -> @ Wo + bo

Sharding: one head per NeuronCore (8 heads, 8 cores). Each core computes
its head's Q/K/V over all tokens, the per-segment attention, and its
head's slice of the output projection (out_h @ Wo[h*64:(h+1)*64, :]).
The host sums the 8 partial projections (bo is added on core 0).

Device layout: token axis is padded per segment to multiples of 128 and
kept TRANSPOSED ([feature, token]) throughout, so every matmul has its
contraction dim on partitions with no on-chip transposes of x. The +pos
fusion happens inside the QK projections by accumulating two matmuls
(W.T@x + W.T@pos) into the same PSUM bank. Softmax denominators come
from a ones-column appended to V. fp32r matmuls (full PE rate at free
dim >= 256, ~1e-4 accuracy).
"""

import os
import sys
import types
import time

import numpy as np
import ml_dtypes

_D = 512
_HD = 64
_H = 8
_S = 196
_NCORES = 8

_prog_cache = {}


def _ensure_ntff_hook():
    """Register the NTFF profile hook that the agent image's antenv lacks."""
    if "antenv.axon_hooks" in sys.modules:
        return
    try:
        from trn_agent_boot.trn_boot import _ntff_profile_via_ctypes

        hook = _ntff_profile_via_ctypes("/opt/axon/libaxon_pjrt.so")
        mod = types.ModuleType("antenv.axon_hooks")
        mod.get_axon_ntff_profile_hook = lambda: hook
        sys.modules["antenv.axon_hooks"] = mod
    except Exception:
        pass


def _segments(channels):
    """Return (seg_len, seg_pad, seg_off, TP) for the padded token axis."""
    seg_len = [int(c) * _S for c in np.asarray(channels).tolist() if int(c) > 0]
    seg_pad = [(l + 127) // 128 * 128 for l in seg_len]
    TP = sum(seg_pad)
    TP = (TP + 511) // 512 * 512
    seg_off = []
    o = 0
    for p in seg_pad:
        seg_off.append(o)
        o += p
    return seg_len, seg_pad, seg_off, TP


def _build_program(seg_key):
    import concourse.bacc as bacc
    import concourse.tile as tile
    from concourse import mybir
    from concourse.masks import make_identity

    seg_len, seg_pad, seg_off, TP = seg_key[0], seg_key[1], seg_key[2], seg_key[3]
    f32 = mybir.dt.float32
    bf16 = mybir.dt.bfloat16

    NTB = TP // 512  # 512-token blocks
    NKT = TP // 128  # 128-token tiles

    nc = bacc.Bacc("TRN2", target_bir_lowering=False, debug=False, num_devices=_NCORES)

    # Inputs: token-major-transposed, block layout [NTB, 512(feature), 512(token)]
    xqb = nc.dram_tensor("xqb", [NTB, _D, 512], bf16, kind="ExternalInput")
    xkb = nc.dram_tensor("xkb", [NTB, _D, 512], bf16, kind="ExternalInput")
    posb = nc.dram_tensor("posb", [NTB, _D, 512], bf16, kind="ExternalInput")
    wq = nc.dram_tensor("wq", [_D, _HD], bf16, kind="ExternalInput")
    wk = nc.dram_tensor("wk", [_D, _HD], bf16, kind="ExternalInput")
    wv = nc.dram_tensor("wv", [_D, _HD], bf16, kind="ExternalInput")
    wo = nc.dram_tensor("wo", [_HD, _D], bf16, kind="ExternalInput")
    qkbias = nc.dram_tensor("qkbias", [_HD, 2], f32, kind="ExternalInput")
    vbias = nc.dram_tensor("vbias", [_HD, 1], f32, kind="ExternalInput")
    obiasT = nc.dram_tensor("obiasT", [128, 4], f32, kind="ExternalInput")
    nseg = len(seg_len)
    padbias = nc.dram_tensor("padbias", [128, nseg], f32, kind="ExternalInput")
    outT = nc.dram_tensor("outT", [_D, TP], f32, kind="ExternalOutput")

    with tile.TileContext(nc) as tc:
        with (
            tc.tile_pool(name="consts", bufs=1) as consts,
            tc.tile_pool(name="persist", bufs=1) as persist,
        ):
            # constants
            wq_sb = consts.tile([128, 4, _HD], bf16)
            wk_sb = consts.tile([128, 4, _HD], bf16)
            wv_sb = consts.tile([128, 4, _HD], bf16)
            nc.sync.dma_start(
                out=wq_sb, in_=wq.rearrange("(k p) d -> p k d", p=128)
            )
            nc.sync.dma_start(
                out=wk_sb, in_=wk.rearrange("(k p) d -> p k d", p=128)
            )
            nc.sync.dma_start(
                out=wv_sb, in_=wv.rearrange("(k p) d -> p k d", p=128)
            )
            wo_sb = consts.tile([_HD, _D], bf16)
            nc.sync.dma_start(out=wo_sb, in_=wo[:, :])
            qkb_sb = consts.tile([_HD, 2], f32)
            nc.sync.dma_start(out=qkb_sb, in_=qkbias[:, :])
            vb_sb = consts.tile([_HD, 1], f32)
            nc.sync.dma_start(out=vb_sb, in_=vbias[:, :])
            ob_sb = consts.tile([128, 4], f32)
            nc.sync.dma_start(out=ob_sb, in_=obiasT[:, :])
            pb_sb = consts.tile([128, nseg], f32)
            nc.sync.dma_start(out=pb_sb, in_=padbias[:, :])
            ident = consts.tile([128, 128], bf16)
            make_identity(nc, ident)

            # persistent per-head tensors
            q_sb = persist.tile([_HD, TP], bf16)  # Q^T
            k_sb = persist.tile([_HD, TP], bf16)  # K^T
            v_sb = persist.tile([128, NKT, _HD + 1], bf16)  # V (normal) + ones col
            attn_sb = persist.tile([_HD, TP], bf16)  # attention output^T (normalized)

            # ---------------- Phase 1: projections ----------------
            with (
                tc.tile_pool(name="xin", bufs=3) as xin,
                tc.tile_pool(name="p1sb", bufs=3) as p1sb,
                tc.tile_pool(name="p1qk", bufs=2, space="PSUM") as p1qk,
                tc.tile_pool(name="p1vt", bufs=2, space="PSUM") as p1vt,
                tc.tile_pool(name="p1tr", bufs=2, space="PSUM") as p1tr,
            ):
                for tb in range(NTB):
                    ts = slice(tb * 512, (tb + 1) * 512)
                    xq_t = xin.tile([128, 4, 512], bf16, tag="xq")
                    xk_t = xin.tile([128, 4, 512], bf16, tag="xk")
                    pos_t = xin.tile([128, 4, 512], bf16, tag="pos")
                    nc.sync.dma_start(
                        out=xq_t,
                        in_=xqb[tb].rearrange("(k p) t -> p k t", p=128),
                    )
                    nc.sync.dma_start(
                        out=xk_t,
                        in_=xkb[tb].rearrange("(k p) t -> p k t", p=128),
                    )
                    nc.sync.dma_start(
                        out=pos_t,
                        in_=posb[tb].rearrange("(k p) t -> p k t", p=128),
                    )

                    # x + pos on VectorE (bf16 4x mode), then single-pass projections
                    xqp_t = xin.tile([128, 4, 512], bf16, tag="xqp")
                    xkp_t = xin.tile([128, 4, 512], bf16, tag="xkp")
                    nc.vector.tensor_add(xqp_t, xq_t, pos_t)
                    nc.vector.tensor_add(xkp_t, xk_t, pos_t)
                    # Q^T and K^T accumulate in separate [64, 512] banks
                    q_ps = p1qk.tile([_HD, 512], f32, tag="qps")
                    k_ps = p1qk.tile([_HD, 512], f32, tag="kps")
                    for c in range(4):
                        nc.tensor.matmul(
                            q_ps, lhsT=wq_sb[:, c], rhs=xqp_t[:, c],
                            start=(c == 0), stop=(c == 3),
                        )
                        nc.tensor.matmul(
                            k_ps, lhsT=wk_sb[:, c], rhs=xkp_t[:, c],
                            start=(c == 0), stop=(c == 3),
                        )
                    # V^T accumulates in a [64, 512] bank
                    vt_ps = p1vt.tile([_HD, 512], f32)
                    for c in range(4):
                        nc.tensor.matmul(
                            vt_ps, lhsT=wv_sb[:, c], rhs=xk_t[:, c],
                            start=(c == 0), stop=(c == 3),
                        )

                    # copy out with bias
                    nc.scalar.activation(
                        out=q_sb[:, ts], in_=q_ps,
                        func=mybir.ActivationFunctionType.Identity,
                        bias=qkb_sb[:, 0:1],
                    )
                    nc.scalar.activation(
                        out=k_sb[:, ts], in_=k_ps,
                        func=mybir.ActivationFunctionType.Identity,
                        bias=qkb_sb[:, 1:2],
                    )
                    vt_sb = p1sb.tile([_HD, 512], bf16, tag="vt")
                    nc.scalar.activation(
                        out=vt_sb, in_=vt_ps,
                        func=mybir.ActivationFunctionType.Identity,
                        bias=vb_sb,
                    )
                    # transpose V^T -> V normal, one 128-token tile at a time
                    for i in range(4):
                        kt = tb * 4 + i
                        tr_ps = p1tr.tile([128, _HD], bf16)
                        nc.tensor.transpose(
                            tr_ps, vt_sb[:, i * 128:(i + 1) * 128], ident[0:64, 0:64]
                        )
                        nc.vector.tensor_copy(
                            out=v_sb[:, kt, 0:_HD], in_=tr_ps
                        )
                    nc.vector.memset(
                        v_sb[:, 4 * tb:4 * (tb + 1), _HD:_HD + 1], 1.0
                    )

            # ---------------- Phase 2: attention ----------------
            max_kt = max(p // 128 for p in seg_pad)
            max_l = max(seg_pad)
            with (
                tc.tile_pool(name="expp", bufs=2) as expp,
                tc.tile_pool(name="p2sb", bufs=3) as p2sb,
                tc.tile_pool(name="p2sc", bufs=3, space="PSUM") as p2sc,
                tc.tile_pool(name="p2o", bufs=2, space="PSUM") as p2o,
            ):
                for s in range(len(seg_len)):
                    off, L, real = seg_off[s], seg_pad[s], seg_len[s]
                    nkt = L // 128
                    qbs = [(qb0, min(512, L - qb0)) for qb0 in range(0, L, 512)]
                    ex = expp.tile([128, max_kt, max_l], bf16, tag="exp")
                    for kt in range(nkt):
                        klo = off + kt * 128
                        for (qb0, qbw) in qbs:
                            sc_ps = p2sc.tile([128, 512], f32, tag="sc")
                            nc.tensor.matmul(
                                sc_ps[:, 0:qbw],
                                lhsT=k_sb[:, klo:klo + 128],
                                rhs=q_sb[:, off + qb0:off + qb0 + qbw],
                                start=True, stop=True,
                            )
                            # padded key rows get bias -87 -> exp ~ 0
                            is_pad_tile = kt == nkt - 1 and real < L
                            nc.scalar.activation(
                                out=ex[:, kt, qb0:qb0 + qbw], in_=sc_ps[:, 0:qbw],
                                func=mybir.ActivationFunctionType.Exp,
                                scale=0.125,
                                bias=pb_sb[:, s:s + 1] if is_pad_tile else 0.0,
                            )
                    for (qb0, qbw) in qbs:
                        o_ps = p2o.tile([_HD + 1, 512], f32, tag="ops")
                        for kt in range(nkt):
                            nc.tensor.matmul(
                                o_ps[:, 0:qbw],
                                lhsT=v_sb[:, off // 128 + kt, :],
                                rhs=ex[:, kt, qb0:qb0 + qbw],
                                start=(kt == 0), stop=(kt == nkt - 1),
                            )
                        sums = p2sb.tile([1, 512], f32, tag="sums")
                        nc.scalar.copy(out=sums[:, 0:qbw], in_=o_ps[_HD:_HD + 1, 0:qbw])
                        bc = p2sb.tile([_HD, 512], f32, tag="bc")
                        nc.gpsimd.partition_broadcast(bc[:, 0:qbw], sums[:, 0:qbw])
                        rec = p2sb.tile([_HD, 512], f32, tag="rec")
                        nc.vector.reciprocal(out=rec[:, 0:qbw], in_=bc[:, 0:qbw])
                        nc.vector.tensor_mul(
                            attn_sb[:, off + qb0:off + qb0 + qbw],
                            o_ps[0:_HD, 0:qbw],
                            rec[:, 0:qbw],
                        )

            # ---------------- Phase 3: output projection ----------------
            with (
                tc.tile_pool(name="p3sb", bufs=3) as p3sb,
                tc.tile_pool(name="p3ps", bufs=3, space="PSUM") as p3ps,
            ):
                for ec in range(4):
                    for tb in range(NTB):
                        ts = slice(tb * 512, (tb + 1) * 512)
                        f_ps = p3ps.tile([128, 512], f32, tag="fin")
                        nc.tensor.matmul(
                            f_ps,
                            lhsT=wo_sb[:, ec * 128:(ec + 1) * 128],
                            rhs=attn_sb[:, ts],
                            start=True, stop=True,
                        )
                        f_sb = p3sb.tile([128, 512], f32, tag="fsb")
                        nc.vector.tensor_scalar_add(
                            f_sb, f_ps, ob_sb[:, ec:ec + 1]
                        )
                        nc.sync.dma_start(
                            out=outT[ec * 128:(ec + 1) * 128, ts], in_=f_sb
                        )

    nc.compile()
    return nc


def _prep_token_major(x, seg_len, seg_pad, seg_off, TP):
    """[T, D] f32 -> blocked transposed [NTB, D, 512] with per-segment padding."""
    xp = np.zeros((TP, _D), dtype=np.float32)
    o = 0
    for l, p, off in zip(seg_len, seg_pad, seg_off):
        xp[off:off + l] = x[o:o + l]
        o += l
    xt = np.ascontiguousarray(xp.T)  # [D, TP]
    return np.ascontiguousarray(
        xt.reshape(_D, TP // 512, 512).transpose(1, 0, 2)
    )  # [NTB, D, 512]


def kernel(x_query, x_keyval, pos, channels, Wq, bq, Wk, bk, Wv, bv, Wo, bo,
           _trace=False, _trace_cores=None):
    _ensure_ntff_hook()
    import concourse.bass_utils as bu

    bu.upload_artifacts = lambda tmpdir: tmpdir  # no S3 egress from here

    x_query = np.asarray(x_query, dtype=np.float32)
    x_keyval = np.asarray(x_keyval, dtype=np.float32)
    pos = np.asarray(pos, dtype=np.float32)
    channels = np.asarray(channels)
    Wq, bq = np.asarray(Wq, np.float32), np.asarray(bq, np.float32)
    Wk, bk = np.asarray(Wk, np.float32), np.asarray(bk, np.float32)
    Wv, bv = np.asarray(Wv, np.float32), np.asarray(bv, np.float32)
    Wo, bo = np.asarray(Wo, np.float32), np.asarray(bo, np.float32)

    C, S, D = x_query.shape
    seg_len, seg_pad, seg_off, TP = _segments(channels)
    assert sum(seg_len) == C * S, "channels inconsistent with batch dim"

    seg_key = (tuple(seg_len), tuple(seg_pad), tuple(seg_off), TP)
    if seg_key not in _prog_cache:
        _prog_cache[seg_key] = _build_program(seg_key)
    nc = _prog_cache[seg_key]

    bf = ml_dtypes.bfloat16
    xqb = _prep_token_major(x_query.reshape(-1, D), seg_len, seg_pad, seg_off, TP).astype(bf)
    xkb = _prep_token_major(x_keyval.reshape(-1, D), seg_len, seg_pad, seg_off, TP).astype(bf)
    posb = _prep_token_major(pos.reshape(-1, D), seg_len, seg_pad, seg_off, TP).astype(bf)

    # per-segment pad bias: -87 on padded key rows of the segment's last k-tile
    nseg = len(seg_len)
    padbias = np.zeros((128, nseg), dtype=np.float32)
    for s in range(nseg):
        plo = seg_len[s] - (seg_pad[s] // 128 - 1) * 128
        if plo < 128:
            padbias[plo:, s] = -87.0

    in_maps = []
    for h in range(_NCORES):
        sl = slice(h * _HD, (h + 1) * _HD)
        ob = bo if h == 0 else np.zeros_like(bo)
        in_maps.append({
            "xqb": xqb,
            "xkb": xkb,
            "posb": posb,
            "wq": np.ascontiguousarray(Wq[:, sl]).astype(bf),
            "wk": np.ascontiguousarray(Wk[:, sl]).astype(bf),
            "wv": np.ascontiguousarray(Wv[:, sl]).astype(bf),
            "wo": np.ascontiguousarray(Wo[sl, :]).astype(bf),
            "qkbias": np.ascontiguousarray(np.stack([bq[sl], bk[sl]], axis=1)),
            "vbias": np.ascontiguousarray(bv[sl].reshape(_HD, 1)),
            "obiasT": np.ascontiguousarray(ob.reshape(4, 128).T),
            "padbias": padbias,
        })

    from concourse.bass_utils import run_bass_kernel_spmd

    kwargs = {}
    if _trace:
        kwargs["trace"] = True
        if _trace_cores is not None:
            kwargs["trace_cores"] = _trace_cores
    res = run_bass_kernel_spmd(nc, in_maps, list(range(_NCORES)), **kwargs)

    acc = np.zeros((_D, TP), dtype=np.float64)
    for h in range(_NCORES):
        acc += res.results[h]["outT"]
    outT = acc.astype(np.float32)

    # unpad + transpose back
    out = np.empty((C * S, D), dtype=np.float32)
    o = 0
    for l, off in zip(seg_len, seg_off):
        out[o:o + l] = outT[:, off:off + l].T
        o += l
    out = out.reshape(C, S, D)

    if _trace:
        kernel._last_exec_time_ns = res.exec_time_ns
        kernel._last_trace = (
            res.instructions_and_trace[1] if res.instructions_and_trace else None
        )
    return out
